# revision 1
# baseline (speedup 1.0000x reference)
"""DCT-blur kernel for 8 Trainium2 NeuronCores.

Computes, per image X [256,256]:
    out = C^T @ (M_b (*) (C @ X @ C^T)) @ C
where C is the orthonormal DCT-II matrix and M_b a per-batch-item
frequency fade mask derived from t[b]:
    sigma = exp(log(.5)(1-t) + log(20)t); tau = sigma^2/2
    fade[i,j] = exp(-(f_i^2+f_j^2) tau);  fade<0.01 -> 0
    M = fade*(1-0.001) + 0.001,   f_i = pi*i/256

Since C is orthonormal, C^T D C = X exactly, so
    out = 0.001*X + 0.999 * C^T @ (fade_clamped (*) (C X C^T)) @ C
and fade_clamped has quarter-disk support: when the support fits the
first 128 frequencies ("sparse" batch items, tau large) stages 2-4
shrink from 4+4+4 to 2+1+2 matmuls.

Sharding: pure data parallel, batch 128 -> 16 per core. The host sorts
batch items by frequency cutoff and deals them round-robin so that all
8 cores see the same per-slot sparse/dense pattern (the Bass program is
specialized per slot; one SPMD program for all cores).

Matmul chain (zero explicit transposes; matmul(out,lhsT,rhs) = lhsT.T@rhs,
contracting the partition dim of both operands; stages 2/4 keep the
constant C stationary so their LDWEIGHTS never waits on an eviction):
    S1  = X.T @ C^T          lhsT=X    rhs=C^T  -> [w,k]
    S2T = C_T.T @ S1         lhsT=C^T  rhs=S1   -> [n,k] = (C X C^T)^T
    S2m = S2T (*) mask       (mask symmetric; DVE, during PSUM eviction)
    S3  = S2mT.T @ C         lhsT=S2m  rhs=C    -> [k,h] = S2m C
    Z   = C.T @ S3           lhsT=C    rhs=S3   -> [w,h] rows-on-partitions
    out = Z + 0.001*X        (DVE stt, during PSUM eviction)
All matmuls are [K<=128, M=128, N=256] float32r (full PE rate at N>=256,
measured rel err ~1.4e-4 per 256-length contraction; plain fp32 is exact
but 4 cycles/row -> 2.3x slower end-to-end, switchable via RDT).
Each stage accumulates into a single [128,512] PSUM bank (4 stage tags x
2 bufs = all 8 banks); evictions are single whole-bank ops: S1/S3 on
ACT, S2m-mask-mul and the final stt on DVE.

The fade mask is separable: fade = u (x) u with u[i]=exp(-f_i^2 tau), so
it is built on-device from t: ACT exp for tau and u (per slot, overlapped
with stage 1), K=1 PE outer product, ACT scaled copy (releases the PSUM
bank early), one all-SBUF DVE threshold op.
"""

from contextlib import ExitStack

import numpy as np

import concourse.bass as bass
import concourse.tile as tile
from concourse import bacc, mybir
from concourse.bass_utils import run_bass_kernel_spmd

B, CH, N = 128, 3, 256
NCORES = 8
BPC = B // NCORES  # batch items (slots) per core
IPC = BPC * CH  # images per core
H = N // 2  # 128 = partition count

MIN_BLUR, MAX_BLUR, MIN_SCALE = 0.5, 20.0, 0.001

F32 = mybir.dt.float32
F32R = mybir.dt.float32r
ALU = mybir.AluOpType
ACTF = mybir.ActivationFunctionType


def build_nc(n_b=BPC, flags=None, rdt=F32R):
    """Build the per-core Bass program.

    n_b: batch items (slots) per core; flags[s]=True -> slot s uses the
    sparse (support < 128 frequencies) path. rdt: dtype fed to the PE
    (float32r = fast/reduced precision, float32 = exact/4x slower).
    """
    if flags is None:
        flags = (False,) * n_b
    assert len(flags) == n_b
    n_img = n_b * CH
    nc = bacc.Bacc(
        "TRN2",
        target_bir_lowering=False,
        debug=False,
        num_devices=NCORES,
    )
    x_d = nc.dram_tensor("x", [n_img, 2, H, N], rdt, kind="ExternalInput").ap()
    t_d = nc.dram_tensor("t", [1, n_b], F32, kind="ExternalInput").ap()
    cm_d = nc.dram_tensor("cm", [2, H, N], rdt, kind="ExternalInput").ap()
    cmt_d = nc.dram_tensor("cmt", [2, H, N], rdt, kind="ExternalInput").ap()
    f2_d = nc.dram_tensor("f2", [1, N], F32, kind="ExternalInput").ap()
    y_d = nc.dram_tensor("y", [n_img, 2, H, N], F32, kind="ExternalOutput").ap()

    # tau = sigma^2/2 = exp(ln(1/8) + 2*ln(40)*t)
    TAU_SCALE = float(2.0 * np.log(MAX_BLUR / MIN_BLUR))
    TAU_BIAS = float(np.log(0.5 * MIN_BLUR * MIN_BLUR))

    with tile.TileContext(nc) as tc, ExitStack() as ctx:
        cpool = ctx.enter_context(tc.tile_pool(name="consts", bufs=1))
        wpool = ctx.enter_context(tc.tile_pool(name="work", bufs=3))
        ppool = ctx.enter_context(tc.tile_pool(name="psum", bufs=2, space="PSUM"))

        # ---- constants into SBUF ----
        # cm_sb[:, kk*256:(kk+1)*256] = C[kk*128:(kk+1)*128, :]  (same for C^T)
        cm_sb = cpool.tile([H, 2 * N], rdt, tag="cm", name="cm_sb")
        cmt_sb = cpool.tile([H, 2 * N], rdt, tag="cmt", name="cmt_sb")
        nc.sync.dma_start(
            cm_sb.rearrange("p (k n) -> p k n", k=2), cm_d.rearrange("k p n -> p k n")
        )
        nc.sync.dma_start(
            cmt_sb.rearrange("p (k n) -> p k n", k=2), cmt_d.rearrange("k p n -> p k n")
        )
        f2_sb = cpool.tile([1, N], F32, tag="f2", name="f2_sb")
        nc.sync.dma_start(f2_sb, f2_d)
        t_sb = cpool.tile([1, n_b], F32, tag="t", name="t_sb")
        nc.sync.dma_start(t_sb, t_d)

        # ---- blur schedule: tau then u rows, all on partition 0 ----
        tbias_sb = cpool.tile([1, 1], F32, tag="tbias", name="tbias_sb")
        nc.vector.memset(tbias_sb, TAU_BIAS)
        tau_sb = cpool.tile([1, n_b], F32, tag="tau", name="tau_sb")
        nc.scalar.activation(tau_sb, t_sb, ACTF.Exp, bias=tbias_sb, scale=TAU_SCALE)
        ntau_sb = cpool.tile([1, n_b], F32, tag="ntau", name="ntau_sb")
        nc.vector.tensor_scalar_mul(ntau_sb, tau_sb, -1.0)
        # u_cat[0, b*N + i] = exp(-f_i^2 * tau_b)   (rdt: feeds PE outer prod)
        u_cat = cpool.tile([1, n_b * N], rdt, tag="ucat", name="u_cat")

        mask_sb = {}  # b -> [128, 256 or 512] tile, 0.999*fade_clamped
        # threshold on the 0.999-scaled fade: fade>=0.01 <=> 0.999*fade>=THR
        THR = float(np.float32(np.float32(1.0 - MIN_SCALE) * np.float32(0.01)))

        def build_mask(b, width):
            # mask[k, kk*N + n] = 0.999*clamp(u[k+kk*H]*u[n]) for k-half kk
            nc.scalar.activation(
                u_cat[0:1, b * N : (b + 1) * N],
                f2_sb,
                ACTF.Exp,
                scale=ntau_sb[0:1, b : b + 1],
            )
            psm = ppool.tile([H, width], F32, tag="ps3", name=f"psm_{b}")
            for kk in range(width // N):
                nc.tensor.matmul(
                    psm[:, kk * N : (kk + 1) * N],
                    u_cat[0:1, b * N + kk * H : b * N + kk * H + H],
                    u_cat[0:1, b * N : (b + 1) * N],
                    start=True,
                    stop=True,
                )
            fade = wpool.tile([H, width], F32, tag="fade", name=f"fade_{b}")
            nc.scalar.activation(fade, psm, ACTF.Copy, scale=1.0 - MIN_SCALE)
            m = cpool.tile([H, width], F32, tag=f"mask_{b}", name=f"mask_{b}")
            # m = (fade_s >= THR) * fade_s, single all-SBUF DVE op
            nc.vector.scalar_tensor_tensor(
                m, fade, THR, fade, op0=ALU.is_ge, op1=ALU.mult
            )
            mask_sb[b] = m

        def c_rhs(kk):
            return cm_sb[:, kk * N : (kk + 1) * N]

        def ct_rhs(kk):
            return cmt_sb[:, kk * N : (kk + 1) * N]

        # ---- main loop: triples = the 3 channels of one slot ----
        for b in range(n_b):
            sparse = flags[b]
            xs, s1, s2, s3 = {}, {}, {}, {}
            for j in range(CH):
                i = b * CH + j
                xf = wpool.tile([H, 2 * N], rdt, tag=f"x{j}", bufs=8, name=f"x_{i}")
                nc.sync.dma_start(
                    xf.rearrange("p (k n) -> p k n", k=2),
                    x_d[i].rearrange("k p n -> p k n"),
                )
                xs[j] = xf
            # stage 1: S1 = X.T @ C^T -> [w, k]; col-block ww = w-half
            for j in range(CH):
                i = b * CH + j
                p1 = ppool.tile([H, 2 * N], F32, tag="ps1", name=f"p1_{i}")
                for m in range(2):
                    for kk in range(2):
                        nc.tensor.matmul(
                            p1[:, m * N : (m + 1) * N],
                            xs[j][:, kk * N + m * H : kk * N + m * H + H],
                            ct_rhs(kk),
                            start=(kk == 0),
                            stop=(kk == 1),
                        )
                s = wpool.tile([H, 2 * N], rdt, tag=f"s1_{j}", name=f"s1_{i}")
                nc.scalar.copy(s, p1)
                s1[j] = s
            build_mask(b, N if sparse else 2 * N)
            # stage 2 (C-stationary): S2T = C_T.T @ S1 -> [n, k]; the
            # constant lhsT means no eviction->LDWEIGHTS serialization.
            # Masked eviction (mask is symmetric, layout unchanged).
            n_m2 = 1 if sparse else 2
            for j in range(CH):
                i = b * CH + j
                p2 = ppool.tile([H, n_m2 * N], F32, tag="ps2", name=f"p2_{i}")
                for m in range(n_m2):
                    for ww in range(2):
                        nc.tensor.matmul(
                            p2[:, m * N : (m + 1) * N],
                            cmt_sb[:, ww * N + m * H : ww * N + m * H + H],
                            s1[j][:, ww * N : (ww + 1) * N],
                            start=(ww == 0),
                            stop=(ww == 1),
                        )
                s = wpool.tile([H, n_m2 * N], rdt, tag=f"s2_{j}", name=f"s2_{i}")
                nc.vector.tensor_mul(s, p2, mask_sb[b])
                s2[j] = s
            # stage 3 (data-stationary): S3 = S2mT.T @ C = S2m @ C -> [k, h]
            # sparse: S2m cols k>=128 are all zero -> single k-tile/K-half.
            n_m3 = 1 if sparse else 2
            n_k3 = 1 if sparse else 2
            for j in range(CH):
                i = b * CH + j
                p3 = ppool.tile([H, n_m3 * N], F32, tag="ps3", name=f"p3_{i}")
                for m in range(n_m3):
                    for nn in range(n_k3):
                        nc.tensor.matmul(
                            p3[:, m * N : (m + 1) * N],
                            s2[j][:, nn * N + m * H : nn * N + m * H + H],
                            c_rhs(nn),
                            start=(nn == 0),
                            stop=(nn == n_k3 - 1),
                        )
                s = wpool.tile([H, n_m3 * N], rdt, tag=f"s3_{j}", name=f"s3_{i}")
                nc.scalar.copy(s, p3)
                s3[j] = s
            # stage 4 (C-stationary): Z = C.T @ S3 -> [w, h]; out = Z + 0.001*X
            n_k4 = 1 if sparse else 2
            for j in range(CH):
                i = b * CH + j
                p4 = ppool.tile([H, 2 * N], F32, tag="ps4", name=f"p4_{i}")
                for m in range(2):
                    for kp in range(n_k4):
                        nc.tensor.matmul(
                            p4[:, m * N : (m + 1) * N],
                            cm_sb[:, kp * N + m * H : kp * N + m * H + H],
                            s3[j][:, kp * N : (kp + 1) * N],
                            start=(kp == 0),
                            stop=(kp == n_k4 - 1),
                        )
                o = wpool.tile([H, 2 * N], F32, tag=f"o{j}", bufs=4, name=f"o_{i}")
                nc.vector.scalar_tensor_tensor(
                    o, xs[j], MIN_SCALE, p4, op0=ALU.mult, op1=ALU.add
                )
                nc.sync.dma_start(
                    y_d[i].rearrange("k p n -> p k n"),
                    o.rearrange("p (k n) -> p k n", k=2),
                )

    nc.compile()
    return nc


def host_constants():
    n = np.arange(N, dtype=np.float64)
    C = np.cos(np.pi * (n[None, :] + 0.5) * n[:, None] / N)
    scale = np.where(n[:, None] == 0, np.sqrt(1.0 / N), np.sqrt(2.0 / N))
    C = (C * scale).astype(np.float32)
    f = (np.pi * np.arange(N) / N).astype(np.float32)
    f2 = (f * f).astype(np.float32)
    return C, f2


def sparse_of_t(t):
    """True where the clamped fade's support fits the first H freqs (with
    a 2-index safety margin)."""
    t64 = np.asarray(t, dtype=np.float64)
    sigma = np.exp(np.log(MIN_BLUR) * (1 - t64) + np.log(MAX_BLUR) * t64)
    tau = sigma * sigma / 2.0
    lim = np.log(100.0) / tau  # keep (i,j) with f_i^2+f_j^2 <= lim
    f126 = (np.pi * (H - 2) / N) ** 2
    return lim < f126


_CACHE = {}


RDT = F32R  # PE dtype: F32R (fast) or F32 (exact)


def _get_nc(flags):
    key = (flags, RDT)
    if key not in _CACHE:
        _CACHE[key] = build_nc(BPC, flags, rdt=RDT)
    return _CACHE[key]


def _run(x, t, trace=False, tmpdir=None):
    x = np.ascontiguousarray(np.asarray(x, dtype=np.float32))
    t = np.asarray(t, dtype=np.float32)
    assert x.shape == (B, CH, N, N) and t.shape == (B,)

    sparse = sparse_of_t(t)
    # Sort sparse items first so the 8 items of each slot share a flag;
    # deal round-robin: slot s of core c gets sorted item s*8+c.
    order = np.argsort(sparse, kind="stable")  # dense first
    flags = tuple(
        bool(sparse[order[s * NCORES : (s + 1) * NCORES]].all()) for s in range(BPC)
    )
    nc = _get_nc(flags)

    C, f2 = host_constants()
    Cc = np.ascontiguousarray(C)
    Ct = np.ascontiguousarray(C.T)
    in_maps = []
    for c in range(NCORES):
        items = order[np.arange(BPC) * NCORES + c]  # slot s -> batch index
        in_maps.append(
            {
                "x": x[items].reshape(IPC, 2, H, N),
                "t": t[items].reshape(1, BPC),
                "cm": Cc.reshape(2, H, N),
                "cmt": Ct.reshape(2, H, N),
                "f2": f2.reshape(1, N),
            }
        )
    res = run_bass_kernel_spmd(
        nc, in_maps, core_ids=list(range(NCORES)), trace=trace, tmpdir=tmpdir
    )
    out = np.empty_like(x)
    for c in range(NCORES):
        items = order[np.arange(BPC) * NCORES + c]
        out[items] = res.results[c]["y"].reshape(BPC, CH, N, N)
    return out, res


def kernel(x, t):
    out, _ = _run(x, t)
    return out


def kernel_with_profile(x, t, tmpdir=None):
    out, res = _run(x, t, trace=True, tmpdir=tmpdir)
    return out, res



# revision 3
# speedup vs baseline: 1.0727x; 1.0727x over previous
"""DCT-blur kernel for 8 Trainium2 NeuronCores.

Computes, per image X [256,256]:
    out = C^T @ (M_b (*) (C @ X @ C^T)) @ C
where C is the orthonormal DCT-II matrix and M_b a per-batch-item
frequency fade mask derived from t[b]:
    sigma = exp(log(.5)(1-t) + log(20)t); tau = sigma^2/2
    fade[i,j] = exp(-(f_i^2+f_j^2) tau);  fade<0.01 -> 0
    M = fade*(1-0.001) + 0.001,   f_i = pi*i/256
Since C is orthonormal the 0.001 floor is pulled out exactly:
    out = 0.001*X + 0.999 * C^T @ (fade_clamped (*) (C X C^T)) @ C

The clamped fade has quarter-disk support of radius s(t) = O(40^-t)
indices.  Per batch item we bake the axis cutoff s (rounded up to 32)
into the program:
  * s <= 128 ("sparse", ~63% of items): all four matmul stages touch
    only the first s frequencies.  Stages 1-2 then have free dim s<256,
    which fp32r runs at 1/4 rate, so the whole sparse path uses bf16
    operands (1 cycle/row at any free dim, and FWL halves LDWEIGHTS).
    The mask filters the bf16 rounding noise, keeping those items'
    error small.
  * s > 128 ("dense"): baseline fp32r 16-matmul path (these items
    dominate the output absmax, so they keep the precise dtype).

Layout: images are stored per-partition as row pairs (partition p holds
rows 2p, 2p+1), so every x / y DMA is a straight [128, 3*512] copy with
2KB-contiguous lines per image (one dma_start per slot of 3 channels).
The DCT constants absorb the permutation: stage-1 rhs is C^T with rows
even/odd-split (ctp), stage-4 weights are C with columns even/odd-split
(cm4), so stage 4 writes PSUM in exactly the row-pair layout and the
final out = Z + 0.001*X DVE op is elementwise aligned with the x tile.
Output is written as bf16 (halves write traffic, 2x DVE rate; ~2e-4
extra rounding vs the 2e-2 gate).

Matmul chain per image (matmul(out,lhsT,rhs) = lhsT.T@rhs, contracting
the partition dim; sparse shapes in brackets):
    S1  = (C X)^T      lhsT=X chunks   rhs=ctp   [128,2s]  4 MM, N=s
    S2T = mask (*) C X C^T (transposed)          [s,s]     2 MM, N=s
    S3  = S2m @ C      lhsT=S2m        rhs=cm    [s,256]   1 MM, N=256
    Z   = C^T @ S3     lhsT=cm4        rhs=S3    [128,512] 2 MM, N=256
    out = Z + 0.001*X  (DVE stt into bf16, during PSUM eviction)

Head: the PE is kept busy from ~0.2us (HAM warm) by junk matmuls plus
prebuilt masks for the dense slots; slots are processed sparsest-first
then dense-descending so the first/last slots are cheap and the big
dense outputs drain mid-kernel.

Sharding: pure data parallel, batch 128 -> 16 slots per core.  The host
sorts batch items by cutoff (descending) and deals them round-robin so
all 8 cores share one SPMD program; slot configs use the group max s.
"""

from contextlib import ExitStack

import numpy as np
import ml_dtypes

import concourse.bass as bass
import concourse.tile as tile
from concourse import bacc, mybir
from concourse.bass_utils import run_bass_kernel_spmd

B, CH, N = 128, 3, 256
NCORES = 8
BPC = B // NCORES  # batch items (slots) per core
H = N // 2  # 128 = partition count
W_IMG = 2 * N  # 512 floats per partition per image (row pair)

MIN_BLUR, MAX_BLUR, MIN_SCALE = 0.5, 20.0, 0.001

F32 = mybir.dt.float32
F32R = mybir.dt.float32r
BF16 = mybir.dt.bfloat16
BF16_NP = ml_dtypes.bfloat16
ALU = mybir.AluOpType
ACTF = mybir.ActivationFunctionType

DENSE = 256  # cfg value marking the fp32r full path

# tau = sigma^2/2 = exp(ln(1/8) + 2*ln(40)*t)
TAU_SCALE = float(2.0 * np.log(MAX_BLUR / MIN_BLUR))
TAU_BIAS = float(np.log(0.5 * MIN_BLUR * MIN_BLUR))
# threshold on the 0.999-scaled fade: fade>=0.01 <=> 0.999*fade>=THR
THR = float(np.float32(np.float32(1.0 - MIN_SCALE) * np.float32(0.01)))


def build_nc(cfg):
    """cfg: per-slot axis cutoff in dealt order; DENSE(256) = fp32r path,
    else s in {32,64,96,128} = bf16 fine-sparse path."""
    n_b = len(cfg)
    dense_slots = [b for b in range(n_b) if cfg[b] == DENSE]
    sparse_slots = [b for b in range(n_b) if cfg[b] != DENSE]
    n32, n16 = len(dense_slots), len(sparse_slots)
    x32_idx = {b: i for i, b in enumerate(dense_slots)}
    x16_idx = {b: i for i, b in enumerate(sparse_slots)}
    # process sparsest slot first (cheap start on bf16 consts), then the
    # dense slots descending, then the remaining sparse slots
    procorder = ([n_b - 1] if cfg[n_b - 1] != DENSE else []) + [
        b for b in range(n_b) if b != n_b - 1
    ]

    nc = bacc.Bacc(
        "TRN2",
        target_bir_lowering=False,
        debug=False,
        num_devices=NCORES,
    )
    if n32:
        x32_d = nc.dram_tensor("x32", [n32, H, CH * W_IMG], F32R, kind="ExternalInput").ap()
        ctp32_d = nc.dram_tensor("ctp32", [H, 2 * N], F32R, kind="ExternalInput").ap()
        cmt32_d = nc.dram_tensor("cmt32", [H, 2 * N], F32R, kind="ExternalInput").ap()
        cm32_d = nc.dram_tensor("cm32", [H, 2 * N], F32R, kind="ExternalInput").ap()
        cm432_d = nc.dram_tensor("cm432", [H, 2 * N], F32R, kind="ExternalInput").ap()
    if n16:
        x16_d = nc.dram_tensor("x16", [n16, H, CH * W_IMG], BF16, kind="ExternalInput").ap()
        ctp16_d = nc.dram_tensor("ctp16", [H, 2 * N], BF16, kind="ExternalInput").ap()
        cmt16_d = nc.dram_tensor("cmt16", [H, 2 * N], BF16, kind="ExternalInput").ap()
        cm16_d = nc.dram_tensor("cm16", [H, 2 * N], BF16, kind="ExternalInput").ap()
        cm416_d = nc.dram_tensor("cm416", [H, 2 * N], BF16, kind="ExternalInput").ap()
    t_d = nc.dram_tensor("t", [1, n_b], F32, kind="ExternalInput").ap()
    f2_d = nc.dram_tensor("f2", [1, N], F32, kind="ExternalInput").ap()
    y_d = nc.dram_tensor("y", [n_b, H, CH * W_IMG], BF16, kind="ExternalOutput").ap()

    with tile.TileContext(nc) as tc, ExitStack() as ctx:
        cpool = ctx.enter_context(tc.tile_pool(name="consts", bufs=1))
        wpool = ctx.enter_context(tc.tile_pool(name="work", bufs=2))
        ppool = ctx.enter_context(tc.tile_pool(name="psum", bufs=2, space="PSUM"))

        # ---- constants into SBUF (straight [128, 512] copies) ----
        t_sb = cpool.tile([1, n_b], F32, tag="t", name="t_sb")
        nc.sync.dma_start(t_sb, t_d)
        f2_sb = cpool.tile([1, N], F32, tag="f2", name="f2_sb")
        nc.sync.dma_start(f2_sb, f2_d)
        if n16:
            ctp16 = cpool.tile([H, 2 * N], BF16, tag="ctp16", name="ctp16")
            nc.sync.dma_start(ctp16, ctp16_d)
            cmt16 = cpool.tile([H, 2 * N], BF16, tag="cmt16", name="cmt16")
            nc.sync.dma_start(cmt16, cmt16_d)
            cm16 = cpool.tile([H, 2 * N], BF16, tag="cm16", name="cm16")
            nc.sync.dma_start(cm16, cm16_d)
            cm416 = cpool.tile([H, 2 * N], BF16, tag="cm416", name="cm416")
            nc.sync.dma_start(cm416, cm416_d)
        if n32:
            ctp32 = cpool.tile([H, 2 * N], F32R, tag="ctp32", name="ctp32")
            nc.sync.dma_start(ctp32, ctp32_d)
            cmt32 = cpool.tile([H, 2 * N], F32R, tag="cmt32", name="cmt32")
            nc.sync.dma_start(cmt32, cmt32_d)
            cm32 = cpool.tile([H, 2 * N], F32R, tag="cm32", name="cm32")
            nc.sync.dma_start(cm32, cm32_d)
            cm432 = cpool.tile([H, 2 * N], F32R, tag="cm432", name="cm432")
            nc.sync.dma_start(cm432, cm432_d)

        # ---- PE warmup: junk matmuls with no data deps (HAM ramp) ----
        wdt = BF16 if n16 else F32R
        wtile = cpool.tile([H, H], wdt, tag="warm", name="warm")
        nc.vector.memset(wtile, 0.0)
        pj = ppool.tile([H, H], F32, tag="ps2", name="pjunk")
        for _ in range(14):
            nc.tensor.matmul(pj, wtile, wtile, start=True, stop=True)

        # ---- blur schedule: tau then -tau, on partition 0 ----
        tbias_sb = cpool.tile([1, 1], F32, tag="tbias", name="tbias_sb")
        nc.vector.memset(tbias_sb, TAU_BIAS)
        tau_sb = cpool.tile([1, n_b], F32, tag="tau", name="tau_sb")
        nc.scalar.activation(tau_sb, t_sb, ACTF.Exp, bias=tbias_sb, scale=TAU_SCALE)
        ntau_sb = cpool.tile([1, n_b], F32, tag="ntau", name="ntau_sb")
        nc.vector.tensor_scalar_mul(ntau_sb, tau_sb, -1.0)

        mask_sb = {}  # slot -> mask tile, 0.999*fade_clamped

        def build_mask(b):
            s = cfg[b]
            if s == DENSE:
                u = wpool.tile([1, N], F32R, tag="u32", name=f"u_{b}")
                nc.scalar.activation(u, f2_sb, ACTF.Exp, scale=ntau_sb[0:1, b : b + 1])
                psm = ppool.tile([H, 2 * N], F32, tag="ps3", name=f"psm_{b}")
                for kk in range(2):
                    nc.tensor.matmul(
                        psm[:, kk * N : (kk + 1) * N],
                        u[0:1, kk * H : kk * H + H],
                        u[0:1, 0:N],
                        start=True,
                        stop=True,
                    )
                fade = wpool.tile([H, 2 * N], F32, tag="fade", name=f"fade_{b}")
                nc.scalar.activation(fade, psm, ACTF.Copy, scale=1.0 - MIN_SCALE)
                m = wpool.tile([H, 2 * N], F32, tag="mask", bufs=n32 + 3, name=f"mask_{b}")
                nc.vector.scalar_tensor_tensor(
                    m, fade, THR, fade, op0=ALU.is_ge, op1=ALU.mult
                )
            else:
                u = wpool.tile([1, s], BF16, tag="u16", name=f"u_{b}")
                nc.scalar.activation(
                    u, f2_sb[0:1, 0:s], ACTF.Exp, scale=ntau_sb[0:1, b : b + 1]
                )
                psm = ppool.tile([H, s], F32, tag="ps3", name=f"psm_{b}")
                nc.tensor.matmul(psm[0:s, :], u, u, start=True, stop=True)
                fade = wpool.tile([H, s], F32, tag="fade", name=f"fade_{b}")
                nc.scalar.activation(
                    fade[0:s, :], psm[0:s, :], ACTF.Copy, scale=1.0 - MIN_SCALE
                )
                m = wpool.tile([H, s], F32, tag="mask", bufs=n32 + 3, name=f"mask_{b}")
                nc.vector.scalar_tensor_tensor(
                    m[0:s, :], fade[0:s, :], THR, fade[0:s, :], op0=ALU.is_ge, op1=ALU.mult
                )
            mask_sb[b] = m

        # masks for the dense slots (and the first-processed slot) are
        # built up front: real PE work during the DMA-bound head, and
        # no mask dependency inside the dense stretch
        prebuilt = []
        if procorder[0] not in dense_slots:
            prebuilt.append(procorder[0])
        prebuilt += dense_slots
        for b in prebuilt:
            build_mask(b)

        def emit_sparse(b):
            s = cfg[b]
            xs = wpool.tile([H, CH * W_IMG], BF16, tag="x16", bufs=6, name=f"x_{b}")
            nc.sync.dma_start(xs, x16_d[x16_idx[b]])
            if b not in mask_sb:
                build_mask(b)
            s1, s2, s3 = {}, {}, {}
            # stage 1: S1 = (C X)^T -> [w, k<s]; m = w-half, r = row parity
            for j in range(CH):
                p1 = ppool.tile([H, 2 * s], F32, tag="ps1", name=f"p1_{b}_{j}")
                for m in range(2):
                    for r in range(2):
                        nc.tensor.matmul(
                            p1[:, m * s : (m + 1) * s],
                            xs[:, j * W_IMG + r * N + m * H : j * W_IMG + r * N + m * H + H],
                            ctp16[:, r * N : r * N + s],
                            start=(r == 0),
                            stop=(r == 1),
                        )
                t1 = wpool.tile([H, 2 * s], BF16, tag=f"s1x{j}", bufs=3, name=f"s1_{b}_{j}")
                nc.scalar.copy(t1, p1)
                s1[j] = t1
            # stage 2 (C-stationary): S2T = masked (C X C^T)^T -> [n<s, k<s]
            for j in range(CH):
                p2 = ppool.tile([H, s], F32, tag="ps2", name=f"p2_{b}_{j}")
                for ww in range(2):
                    nc.tensor.matmul(
                        p2[0:s, :],
                        cmt16[:, ww * N : ww * N + s],
                        s1[j][:, ww * s : (ww + 1) * s],
                        start=(ww == 0),
                        stop=(ww == 1),
                    )
                t2 = wpool.tile([H, s], BF16, tag=f"s2x{j}", bufs=3, name=f"s2_{b}_{j}")
                nc.vector.tensor_mul(t2[0:s, :], p2[0:s, :], mask_sb[b][0:s, :])
                s2[j] = t2
            # stage 3 (data-stationary): S3 = S2m @ C -> [k<s, h]
            for j in range(CH):
                p3 = ppool.tile([H, N], F32, tag="ps3", name=f"p3_{b}_{j}")
                nc.tensor.matmul(
                    p3[0:s, :], s2[j][0:s, :], cm16[0:s, 0:N], start=True, stop=True
                )
                t3 = wpool.tile([H, N], BF16, tag=f"s3x{j}", bufs=3, name=f"s3_{b}_{j}")
                nc.scalar.copy(t3[0:s, :], p3[0:s, :])
                s3[j] = t3
            # stage 4 (C-stationary): Z = C^T @ S3 in row-pair layout
            o = wpool.tile([H, CH * W_IMG], BF16, tag="o", bufs=3, name=f"o_{b}")
            for j in range(CH):
                p4 = ppool.tile([H, 2 * N], F32, tag="ps4", name=f"p4_{b}_{j}")
                for m in range(2):
                    nc.tensor.matmul(
                        p4[:, m * N : (m + 1) * N],
                        cm416[0:s, m * H : m * H + H],
                        s3[j][0:s, :],
                        start=True,
                        stop=True,
                    )
                nc.vector.scalar_tensor_tensor(
                    o[:, j * W_IMG : (j + 1) * W_IMG],
                    xs[:, j * W_IMG : (j + 1) * W_IMG],
                    MIN_SCALE,
                    p4,
                    op0=ALU.mult,
                    op1=ALU.add,
                )
            nc.sync.dma_start(y_d[b], o)

        def emit_dense(b):
            xs = wpool.tile([H, CH * W_IMG], F32R, tag="x32", bufs=3, name=f"x_{b}")
            nc.sync.dma_start(xs, x32_d[x32_idx[b]])
            s1, s2, s3 = {}, {}, {}
            for j in range(CH):
                p1 = ppool.tile([H, 2 * N], F32, tag="ps1", name=f"p1_{b}_{j}")
                for m in range(2):
                    for r in range(2):
                        nc.tensor.matmul(
                            p1[:, m * N : (m + 1) * N],
                            xs[:, j * W_IMG + r * N + m * H : j * W_IMG + r * N + m * H + H],
                            ctp32[:, r * N : (r + 1) * N],
                            start=(r == 0),
                            stop=(r == 1),
                        )
                t1 = wpool.tile([H, 2 * N], F32R, tag=f"s1d{j}", bufs=3, name=f"s1_{b}_{j}")
                nc.scalar.copy(t1, p1)
                s1[j] = t1
            for j in range(CH):
                p2 = ppool.tile([H, 2 * N], F32, tag="ps2", name=f"p2_{b}_{j}")
                for m2 in range(2):
                    for ww in range(2):
                        nc.tensor.matmul(
                            p2[:, m2 * N : (m2 + 1) * N],
                            cmt32[:, ww * N + m2 * H : ww * N + m2 * H + H],
                            s1[j][:, ww * N : (ww + 1) * N],
                            start=(ww == 0),
                            stop=(ww == 1),
                        )
                t2 = wpool.tile([H, 2 * N], F32R, tag=f"s2d{j}", bufs=3, name=f"s2_{b}_{j}")
                nc.vector.tensor_mul(t2, p2, mask_sb[b])
                s2[j] = t2
            for j in range(CH):
                p3 = ppool.tile([H, 2 * N], F32, tag="ps3", name=f"p3_{b}_{j}")
                for m3 in range(2):
                    for nn in range(2):
                        nc.tensor.matmul(
                            p3[:, m3 * N : (m3 + 1) * N],
                            s2[j][:, nn * N + m3 * H : nn * N + m3 * H + H],
                            cm32[:, nn * N : (nn + 1) * N],
                            start=(nn == 0),
                            stop=(nn == 1),
                        )
                t3 = wpool.tile([H, 2 * N], F32R, tag=f"s3d{j}", bufs=3, name=f"s3_{b}_{j}")
                nc.scalar.copy(t3, p3)
                s3[j] = t3
            o = wpool.tile([H, CH * W_IMG], BF16, tag="o", bufs=3, name=f"o_{b}")
            for j in range(CH):
                p4 = ppool.tile([H, 2 * N], F32, tag="ps4", name=f"p4_{b}_{j}")
                for m in range(2):
                    for kp in range(2):
                        nc.tensor.matmul(
                            p4[:, m * N : (m + 1) * N],
                            cm432[:, kp * N + m * H : kp * N + m * H + H],
                            s3[j][:, kp * N : (kp + 1) * N],
                            start=(kp == 0),
                            stop=(kp == 1),
                        )
                nc.vector.scalar_tensor_tensor(
                    o[:, j * W_IMG : (j + 1) * W_IMG],
                    xs[:, j * W_IMG : (j + 1) * W_IMG],
                    MIN_SCALE,
                    p4,
                    op0=ALU.mult,
                    op1=ALU.add,
                )
            nc.sync.dma_start(y_d[b], o)

        for b in procorder:
            if cfg[b] == DENSE:
                emit_dense(b)
            else:
                emit_sparse(b)

    nc.compile()
    return nc


def host_constants():
    n = np.arange(N, dtype=np.float64)
    C = np.cos(np.pi * (n[None, :] + 0.5) * n[:, None] / N)
    scale = np.where(n[:, None] == 0, np.sqrt(1.0 / N), np.sqrt(2.0 / N))
    C = (C * scale).astype(np.float32)
    f = (np.pi * np.arange(N) / N).astype(np.float32)
    f2 = (f * f).astype(np.float32)
    A = np.ascontiguousarray(C.T)  # A[h, k] = C[k, h]
    # ctp[p, r*N+k] = C[k, 2p+r]          (stage-1 rhs, rows even/odd split)
    ctp = A.reshape(H, 2, N).reshape(H, 2 * N)
    # cmt[p, ww*N+n] = C[n, ww*128+p]     (stage-2 weights, natural w-halves)
    cmt = A.reshape(2, H, N).transpose(1, 0, 2).reshape(H, 2 * N)
    # cm[p, nn*N+h] = C[nn*128+p, h]      (stage-3 rhs, natural rows)
    cm = C.reshape(2, H, N).transpose(1, 0, 2).reshape(H, 2 * N)
    # cm4[p, kp*N+m*H+w] = C[kp*128+p, 2w+m]  (stage-4 weights, cols split)
    cm4 = C.reshape(2, H, H, 2).transpose(1, 0, 3, 2).reshape(H, 2 * N)
    return (
        np.ascontiguousarray(ctp),
        np.ascontiguousarray(cmt),
        np.ascontiguousarray(cm),
        np.ascontiguousarray(cm4),
        f2,
    )


def s_of_t(t):
    """Per-item axis cutoff: smallest s (with +2 safety margin) such that
    every kept fade entry (>=0.01) has both indices < s."""
    t64 = np.asarray(t, dtype=np.float64)
    sigma = np.exp(np.log(MIN_BLUR) * (1 - t64) + np.log(MAX_BLUR) * t64)
    tau = sigma * sigma / 2.0
    lim = np.log(100.0) / tau  # keep (i,j) with f_i^2+f_j^2 <= lim
    imax = np.floor(N * np.sqrt(lim) / np.pi).astype(np.int64)
    return np.minimum(imax + 3, N).astype(np.int64)


def interleave(img):
    """[3,256,256] fp32 -> [128, 3*512] row-pair layout."""
    return np.ascontiguousarray(
        img.reshape(CH, H, 2, N).transpose(1, 0, 2, 3).reshape(H, CH * W_IMG)
    )


def deinterleave(arr):
    """[128, 3*512] -> [3,256,256]."""
    return arr.reshape(H, CH, 2, N).transpose(1, 0, 2, 3).reshape(CH, N, N)


_CACHE = {}


def _get_nc(cfg):
    if cfg not in _CACHE:
        _CACHE[cfg] = build_nc(cfg)
    return _CACHE[cfg]


def _run(x, t, trace=False, tmpdir=None):
    x = np.ascontiguousarray(np.asarray(x, dtype=np.float32))
    t = np.asarray(t, dtype=np.float32)
    assert x.shape == (B, CH, N, N) and t.shape == (B,)

    s_item = s_of_t(t)
    # sort densest first, deal round-robin: slot b of core c gets item
    # order[b*8+c]; slot config = group max (sorted -> first of group)
    order = np.argsort(-s_item, kind="stable")
    cfg = []
    for b in range(BPC):
        smax = int(s_item[order[b * NCORES]])
        cfg.append(DENSE if smax > H else int(min(H, ((smax + 31) // 32) * 32)))
    cfg = tuple(cfg)
    nc = _get_nc(cfg)

    ctp, cmt, cm, cm4, f2 = host_constants()
    n32 = sum(1 for s in cfg if s == DENSE)
    in_maps = []
    for c in range(NCORES):
        items = order[np.arange(BPC) * NCORES + c]  # slot b -> batch index
        x32 = np.empty((n32, H, CH * W_IMG), np.float32) if n32 else None
        x16 = (
            np.empty((BPC - n32, H, CH * W_IMG), BF16_NP) if n32 < BPC else None
        )
        for b in range(BPC):
            il = interleave(x[items[b]])
            if cfg[b] == DENSE:
                x32[b] = il
            else:
                x16[b - n32] = il.astype(BF16_NP)
        m = {
            "t": np.ascontiguousarray(t[items].reshape(1, BPC)),
            "f2": f2.reshape(1, N),
        }
        if n32:
            m["x32"] = x32
            m["ctp32"], m["cmt32"], m["cm32"], m["cm432"] = ctp, cmt, cm, cm4
        if n32 < BPC:
            m["x16"] = x16
            m["ctp16"] = ctp.astype(BF16_NP)
            m["cmt16"] = cmt.astype(BF16_NP)
            m["cm16"] = cm.astype(BF16_NP)
            m["cm416"] = cm4.astype(BF16_NP)
        in_maps.append(m)
    res = run_bass_kernel_spmd(
        nc, in_maps, core_ids=list(range(NCORES)), trace=trace, tmpdir=tmpdir
    )
    out = np.empty_like(x)
    for c in range(NCORES):
        items = order[np.arange(BPC) * NCORES + c]
        y = res.results[c]["y"].astype(np.float32)
        for b in range(BPC):
            out[items[b]] = deinterleave(y[b])
    return out, res


def kernel(x, t):
    out, _ = _run(x, t)
    return out


def kernel_with_profile(x, t, tmpdir=None):
    out, res = _run(x, t, trace=True, tmpdir=tmpdir)
    return out, res
